# revision 1
# baseline (speedup 1.0000x reference)
"""BPCA pooling kernel for Trainium2 (Bass/Tile), 8-core data-parallel.

Per core: 4 images [128,128,64] f32.
  1. DMA each image HBM->SBUF in a (i,jh)-partition layout so that the
     2x2->depth output permutation becomes a pure AP trick.
  2. Raw 4x4 Gram per image on PE via the diagonal-block trick:
     accumulate X_chunk^T @ X_chunk ([128,128] PSUM, 64 chunks); the 32
     diagonal 4x4 blocks sum to the Gram over all rows.
  3. Column sums on ACT (identity-activation with accum_out).
  4. Fold: mask off-diagonal blocks (DVE), E^T-matmul collapses the 32
     diag blocks + partition-sums of the ACT accums, small DVE reduces.
  5. Standardize Gram -> Ghat; batched parallel-Jacobi (3 sweeps x 3
     disjoint-pair rounds) on DVE+ACT for all 4 images at once; pick top
     eigencolumn, fix sign, form per-image weights w[4] and bias b.
  6. Projection pass: out = sum_a w_a * x_a + b as 4 DVE ops per image,
     written in output order, DMA to HBM.
"""

import sys
from contextlib import ExitStack

import numpy as np

for _p in ("/opt/trn_rl_repo",):
    if _p not in sys.path:
        sys.path.insert(0, _p)

import concourse.bass as bass
import concourse.bacc as bacc
import concourse.tile as tile
from concourse import mybir
from concourse.bass_utils import run_bass_kernel_spmd

AF = mybir.ActivationFunctionType
OP = mybir.AluOpType
AX = mybir.AxisListType
F32 = mybir.dt.float32
U32 = mybir.dt.uint32

B, H, W, C = 32, 128, 128, 64
NCORES = 8
IMGS = B // NCORES  # 4 images per core
NROWS = float(H * W * C // 4)  # 262144 rows per image
SQRTN = float(np.sqrt(NROWS))
FREE = H * W * C // 128  # 8192 f32 per partition per image
NCHUNK = FREE // 128  # 64 gram chunks per image


def _emit(ctx, tc, y, x, maskc, em5c, dbg=None):
    nc = tc.nc
    v = nc.vector
    act = nc.scalar

    consts = ctx.enter_context(tc.tile_pool(name="consts", bufs=1))
    xpool = ctx.enter_context(tc.tile_pool(name="xdata", bufs=1))
    gpool = ctx.enter_context(tc.tile_pool(name="gram", bufs=2, space="PSUM"))
    p2pool = ctx.enter_context(tc.tile_pool(name="ps2", bufs=2, space="PSUM"))
    spool = ctx.enter_context(tc.tile_pool(name="small", bufs=1))
    jpool = ctx.enter_context(tc.tile_pool(name="jac", bufs=2))
    rpool = ctx.enter_context(tc.tile_pool(name="res", bufs=2))
    dpool = ctx.enter_context(tc.tile_pool(name="dscratch", bufs=1, space="DRAM"))

    mask = consts.tile([128, 128], F32)
    nc.sync.dma_start(mask[:], maskc[:])
    em5 = consts.tile([128, 5], F32)
    nc.sync.dma_start(em5[:], em5c[:])

    # ---- load all images; emit ACT sums as data lands --------------------
    xt = []
    sacc = spool.tile([128, 8 * IMGS], F32)  # (img, half, class) sums
    for i in range(IMGS):
        xi = xpool.tile([128, FREE], F32, name=f"ximg{i}")
        xt.append(xi)
        # SBUF: partition (i64, jh2); free (dh2, wl64, c64)
        src5 = (
            x[i]
            .rearrange("(i dh) (jh wl) c -> i dh jh wl c", dh=2, jh=2)
            .transpose([0, 2, 1, 3, 4])  # [i, jh, dh, wl, c]
        )
        for dh in range(2):
            for wh in range(2):
                f0 = dh * 4096 + wh * 2048
                nc.sync.dma_start(
                    xi[:, f0 : f0 + 2048],
                    src5[:, :, dh, wh * 32 : (wh + 1) * 32, :],
                )
    scratch = spool.tile([128, 1024], F32)  # identity outs (discarded)
    for i in range(IMGS):
        for hf in range(2):
            pv = (
                xt[i][:, hf * 4096 : (hf + 1) * 4096]
                .rearrange("p (f a) -> p f a", a=4)
                .transpose([0, 2, 1])
            )
            col = 8 * i + 4 * hf
            for a in range(4):
                sl = sacc[:, col + a : col + a + 1]
                if (i + hf) % 2 == 0:
                    act.activation(
                        scratch[:], pv[:, a, :], AF.Identity, accum_out=sl
                    )
                else:
                    v.tensor_scalar(
                        scratch[:], pv[:, a, :], 1.0, 0.0, OP.mult, OP.add,
                        accum_out=sl,
                    )

    # ---- per-image Gram on PE -------------------------------------------
    gps = []
    for i in range(IMGS):
        gp = gpool.tile([128, 128], F32, name=f"gram{i}", tag="gram")
        gps.append(gp)
        for k in range(NCHUNK):
            chunk = xt[i][:, k * 128 : (k + 1) * 128]
            nc.tensor.matmul(
                gp[:], chunk, chunk, start=(k == 0), stop=(k == NCHUNK - 1)
            )

    # ---- fold: mask + E-matmul + reduces --------------------------------
    G4 = spool.tile([5, 16], F32)  # [a, (img, b)] raw Gram after j-reduce
    S1b = spool.tile([5, 16], F32)  # row 4 = sums; cols (img, a)
    for i in range(IMGS):
        gs = jpool.tile([128, 128], F32, tag="gs")
        v.tensor_copy(gs[:], gps[i][:])
        gm = jpool.tile([128, 128], F32, tag="gm")
        v.tensor_tensor(gm[:], gs[:], mask[:], OP.mult)
        ps2 = p2pool.tile([5, 136], F32, tag="ps2", name=f"ps2_{i}")
        nc.tensor.matmul(ps2[:, :128], em5[:], gm[:], start=True, stop=True)
        nc.tensor.matmul(
            ps2[:, 128:136],
            em5[:],
            sacc[:, 8 * i : 8 * i + 8],
            start=True,
            stop=True,
        )
        f5 = jpool.tile([5, 136], F32, tag="f5")
        v.tensor_copy(f5[:], ps2[:])
        # sum the 32 diagonal blocks: view [5, (b 4 s1), (j 32 s4)]
        gv = f5[:, :128].rearrange("p (j b) -> p j b", b=4).transpose([0, 2, 1])
        v.tensor_reduce(G4[:, 4 * i : 4 * i + 4], gv, AX.X, OP.add)
        sv = (
            f5[:, 128:136]
            .rearrange("p (pc a) -> p pc a", a=4)
            .transpose([0, 2, 1])
        )
        v.tensor_reduce(S1b[:, 4 * i : 4 * i + 4], sv, AX.X, OP.add)

    # ---- transpose folds into image-on-partition layout ------------------
    A0 = spool.tile([4, 16], F32)  # [img, (a,b)] raw Gram
    S1 = spool.tile([4, 4], F32)  # [img, a] col sums
    dbounce = dpool.tile([4, 16], F32)
    # write G4 (iteration order a, i, b) into DRAM at img-major addresses
    nc.sync.dma_start(
        dbounce[:].rearrange("i (a b) -> i a b", a=4).transpose([1, 0, 2]),
        G4[0:4, :],
    )
    nc.sync.dma_start(A0[:], dbounce[:])
    nc.sync.dma_start(
        S1[:], S1b[4:5, :].rearrange("o (i a) -> o i a", i=4)
    )

    # ---- Ghat = D (G - N m m^T) D --------------------------------------
    jt = lambda shape, tag: jpool.tile(shape, F32, tag=tag, name=tag)
    m = jt([4, 4], "m_mean")
    v.tensor_scalar(m[:], S1[:], 1.0 / NROWS, None, OP.mult)
    mm = jt([4, 16], "mm_outer")
    ma = m[:].unsqueeze(2).broadcast_to([4, 4, 4])
    mb = m[:].unsqueeze(1).broadcast_to([4, 4, 4])
    v.tensor_tensor(mm[:].rearrange("i (a b) -> i a b", a=4), ma, mb, OP.mult)
    Ac = jt([4, 16], "A_cov")
    v.scalar_tensor_tensor(Ac[:], mm[:], -NROWS, A0[:], OP.mult, OP.add)
    vd = Ac[:, 0:16:5]  # diag [4,4]
    sqd = jt([4, 4], "sqd")
    act.activation(sqd[:], vd, AF.Sqrt)
    sqc = jt([4, 4], "sqc")
    v.tensor_scalar(sqc[:], sqd[:], 1e-30, None, OP.max)
    rv0 = jt([4, 4], "rv0")
    v.reciprocal(rv0[:], sqc[:])
    ud = jt([4, 4], "ud")
    v.tensor_tensor(ud[:], vd, rv0[:], OP.mult)
    s2d = jt([4, 4], "s2d")
    v.tensor_tensor(s2d[:], sqc[:], ud[:], OP.add)  # 2*sqrt(Cov_aa), refined
    rv = jt([4, 4], "rv")
    v.reciprocal(rv[:], s2d[:])  # rinv_true / 2
    mk = jt([4, 4], "mk")
    v.tensor_scalar(mk[:], vd, 0.0, None, OP.is_gt)
    rinv = jt([4, 4], "rinv")
    v.tensor_tensor(rinv[:], rv[:], mk[:], OP.mult)
    rr = jt([4, 16], "rr")
    ra = rinv[:].unsqueeze(2).broadcast_to([4, 4, 4])
    rb = rinv[:].unsqueeze(1).broadcast_to([4, 4, 4])
    v.tensor_tensor(rr[:].rearrange("i (a b) -> i a b", a=4), ra, rb, OP.mult)
    A = jt([4, 16], "A_jac")
    v.tensor_tensor(A[:], Ac[:], rr[:], OP.mult)

    # ---- top eigenvector via power-iteration-with-squaring (DVE) --------
    # B^(2^k) -> lam^m v v^T; computed per image inside its own partition:
    # B.B = sum_j col_j x row_j with free-dim broadcasts. Normalize by
    # B[0,0] (a plain per-partition scalar in this layout) every 4 steps.
    NSQ = 15
    Ball = A
    for k in range(NSQ):
        m_ = []
        for j in range(4):
            colv = Ball[:, j:16:4].unsqueeze(2).broadcast_to([4, 4, 4])
            rowv = Ball[:, 4 * j : 4 * j + 4].unsqueeze(1).broadcast_to([4, 4, 4])
            mj = jpool.tile([4, 16], F32, tag=f"sqm{j}", name=f"sqm{j}_{k}")
            v.tensor_tensor(mj[:].rearrange("i (r c) -> i r c", r=4), colv, rowv, OP.mult)
            m_.append(mj)
        a01 = jpool.tile([4, 16], F32, tag="sqa0", name=f"sqa0_{k}")
        v.tensor_tensor(a01[:], m_[0][:], m_[1][:], OP.add)
        a23 = jpool.tile([4, 16], F32, tag="sqa1", name=f"sqa1_{k}")
        v.tensor_tensor(a23[:], m_[2][:], m_[3][:], OP.add)
        Bn = jpool.tile([4, 16], F32, tag="Bsq", name=f"Bsq{k}")
        v.tensor_tensor(Bn[:], a01[:], a23[:], OP.add)
        if (k + 1) % 4 == 0 or k == NSQ - 1:
            dmx = jpool.tile([4, 1], F32, tag="dmx", name=f"dmx_{k}")
            v.tensor_reduce(dmx[:], Bn[:, 0:16:5], AX.X, OP.max)
            r00 = jpool.tile([4, 1], F32, tag="r00", name=f"r00_{k}")
            v.reciprocal(r00[:], dmx[:])
            Bm = jpool.tile([4, 16], F32, tag="Bsq", name=f"Bsqn{k}")
            v.tensor_scalar(Bm[:], Bn[:], r00[:], None, OP.mult)
            Ball = Bm
        else:
            Ball = Bn
    Bf = Ball

    # ---- select max-norm column, sign fix, normalize, w/b ---------------
    Bf3 = Bf[:].rearrange("i (r c) -> i r c", r=4)
    sqB = jt([4, 16], "sqB")
    v.tensor_tensor(sqB[:], Bf[:], Bf[:], OP.mult)
    cn = jt([4, 4], "cn")
    v.tensor_reduce(
        cn[:], sqB[:].rearrange("i (r c) -> i r c", r=4).transpose([0, 2, 1]),
        AX.X, OP.add,
    )
    cmax = jt([4, 1], "cmax")
    v.tensor_reduce(cmax[:], cn[:], AX.X, OP.max)
    km = jt([4, 4], "km")
    v.tensor_scalar(km[:], cn[:], cmax[:], None, OP.is_ge)
    kmb = km[:].unsqueeze(1).broadcast_to([4, 4, 4])
    vm = jt([4, 16], "vm")
    v.tensor_tensor(vm[:].rearrange("i (r c) -> i r c", r=4), Bf3, kmb, OP.mult)
    v4 = jt([4, 4], "v4")
    v.tensor_reduce(v4[:], vm[:].rearrange("i (r c) -> i r c", r=4), AX.X, OP.add)
    sv = jt([4, 1], "sv")
    v.tensor_reduce(sv[:], v4[:], AX.X, OP.add)
    sg = jt([4, 1], "sg")
    v.tensor_scalar(sg[:], sv[:], 0.0, 2.0, OP.is_ge, OP.mult)
    sg2 = jt([4, 1], "sg2")
    v.tensor_scalar(sg2[:], sg[:], 1.0, None, OP.subtract)
    v4s = jt([4, 4], "v4s")
    v.tensor_scalar(v4s[:], v4[:], sg2[:], None, OP.mult)
    # normalize: rn = 1/(2*||v4s||) via ACT sqrt + one Newton step
    vsq = jt([4, 4], "vsq")
    v.tensor_tensor(vsq[:], v4s[:], v4s[:], OP.mult)
    n2 = jt([4, 1], "n2")
    v.tensor_reduce(n2[:], vsq[:], AX.X, OP.add)
    s0n = jt([4, 1], "s0n")
    act.activation(s0n[:], n2[:], AF.Sqrt)
    s0nc = jt([4, 1], "s0nc")
    v.tensor_scalar(s0nc[:], s0n[:], 1e-30, None, OP.max)
    r0n = jt([4, 1], "r0n")
    v.reciprocal(r0n[:], s0nc[:])
    un = jt([4, 1], "un")
    v.tensor_tensor(un[:], n2[:], r0n[:], OP.mult)
    s2n = jt([4, 1], "s2n")
    v.tensor_tensor(s2n[:], s0nc[:], un[:], OP.add)
    rn = jt([4, 1], "rn")
    v.reciprocal(rn[:], s2n[:])
    vw = jt([4, 4], "vw")
    v.tensor_scalar(vw[:], v4s[:], rn[:], None, OP.mult)  # = v_unit/2
    w4 = jt([4, 4], "w4")
    v.scalar_tensor_tensor(w4[:], vw[:], 4.0 * SQRTN, rinv[:], OP.mult, OP.mult)
    wm = jt([4, 4], "wm")
    v.tensor_tensor(wm[:], w4[:], m[:], OP.mult)
    bs = jt([4, 1], "bs")
    v.tensor_reduce(bs[:], wm[:], AX.X, OP.add)
    bneg = jt([4, 1], "bneg")
    v.tensor_scalar(bneg[:], bs[:], -1.0, None, OP.mult)
    wb5 = jt([4, 5], "wb5")
    v.tensor_copy(wb5[:, 0:4], w4[:])
    v.tensor_copy(wb5[:, 4:5], bneg[:])
    wrow = spool.tile([1, 20], F32)
    nc.sync.dma_start(wrow[:], wb5[:])
    wbc = spool.tile([128, 20], F32)
    nc.sync.dma_start(
        wbc[:], wrow[:].unsqueeze(1).broadcast_to([1, 128, 20])
    )

    if dbg is not None:
        nc.sync.dma_start(dbg[:, 0:16], A0[:])
        nc.sync.dma_start(dbg[:, 16:20], S1[:])
        nc.sync.dma_start(dbg[:, 20:36], A[:])
        nc.sync.dma_start(dbg[:, 36:41], wb5[:])
        nc.sync.dma_start(dbg[:, 41:45], v4[:])
        nc.sync.dma_start(dbg[:, 57:73], Ac[:])

    # ---- projection pass + output DMA -----------------------------------
    for i in range(IMGS):
        w_ = lambda a: wbc[:, 5 * i + a : 5 * i + a + 1]
        resT = rpool.tile([128, 2048], F32, tag="resT", name=f"resT{i}")
        rv5 = resT[:].rearrange("p (jl dhh dk) -> p jl dhh dk", jl=32, dhh=2)
        for dh in range(2):
            half = xt[i][:, dh * 4096 : (dh + 1) * 4096].rearrange(
                "p (f a) -> p f a", a=4
            )
            outv = rv5[:, :, dh, :]
            if i < 2:
                # DVE chain: fused multiply-adds on strided reads
                h0 = rpool.tile([128, 1024], F32, tag="p2h0", bufs=1, name=f"h0_{i}_{dh}")
                v.tensor_scalar(h0[:], half[:, :, 0], w_(0), w_(4), OP.mult, OP.add)
                h1 = rpool.tile([128, 1024], F32, tag="p2h1", bufs=1, name=f"h1_{i}_{dh}")
                v.scalar_tensor_tensor(h1[:], half[:, :, 1], w_(1), h0[:], OP.mult, OP.add)
                h2 = rpool.tile([128, 1024], F32, tag="p2h2", bufs=1, name=f"h2_{i}_{dh}")
                v.scalar_tensor_tensor(h2[:], half[:, :, 2], w_(2), h1[:], OP.mult, OP.add)
                v.scalar_tensor_tensor(outv, half[:, :, 3], w_(3), h2[:], OP.mult, OP.add)
            else:
                # ACT does the strided scaled reads; DVE adds contiguous
                ms = []
                for a in range(4):
                    mt = rpool.tile([128, 1024], F32, tag=f"p2m{a}", bufs=1, name=f"m{a}_{i}_{dh}")
                    act.activation(
                        mt[:], half[:, :, a], AF.Identity,
                        bias=w_(4) if a == 0 else 0.0, scale=w_(a),
                    )
                    ms.append(mt)
                a01 = rpool.tile([128, 1024], F32, tag="p2a0", bufs=1, name=f"a01_{i}_{dh}")
                nc.gpsimd.tensor_tensor(a01[:], ms[0][:], ms[1][:], OP.add)
                a23 = rpool.tile([128, 1024], F32, tag="p2a1", bufs=1, name=f"a23_{i}_{dh}")
                nc.gpsimd.tensor_tensor(a23[:], ms[2][:], ms[3][:], OP.add)
                nc.gpsimd.tensor_tensor(outv, a01[:], a23[:], OP.add)
        dst = y[i].rearrange("i2 (jh jl) c -> (i2 jh) (jl c)", jh=2)
        nc.sync.dma_start(dst, resT[:])


_CACHE = {}


def _build(dbg_mode=False):
    key = "nc_dbg" if dbg_mode else "nc"
    if key in _CACHE:
        return _CACHE[key]
    nc = bacc.Bacc("TRN2", target_bir_lowering=False, debug=False)
    x = nc.dram_tensor("x", [IMGS, H, W, C], F32, kind="ExternalInput").ap()
    maskc = nc.dram_tensor("maskc", [128, 128], F32, kind="ExternalInput").ap()
    em5c = nc.dram_tensor("em5c", [128, 5], F32, kind="ExternalInput").ap()
    y = nc.dram_tensor("y", [IMGS, H // 2, W // 2, C], F32, kind="ExternalOutput").ap()
    dbg = (
        nc.dram_tensor("dbg", [4, 73], F32, kind="ExternalOutput").ap()
        if dbg_mode
        else None
    )
    with tile.TileContext(nc) as tc, ExitStack() as ctx:
        _emit(ctx, tc, y, x, maskc, em5c, dbg)
    nc.compile()
    _CACHE[key] = nc
    return nc


def _consts():
    if "mask" not in _CACHE:
        j = np.arange(128)
        blk = (j[:, None] // 4) == (j[None, :] // 4)
        _CACHE["mask"] = blk.astype(np.float32)
        em = np.zeros((128, 5), dtype=np.float32)
        em[j, j % 4] = 1.0
        em[:, 4] = 1.0
        _CACHE["em5"] = em
    return _CACHE["mask"], _CACHE["em5"]


def kernel(inputs: np.ndarray, _trace: bool = False):
    x = np.ascontiguousarray(np.asarray(inputs, dtype=np.float32))
    assert x.shape == (B, H, W, C), x.shape
    nc = _build()
    mask, em5 = _consts()
    in_maps = [
        {"x": x[i * IMGS : (i + 1) * IMGS], "maskc": mask, "em5c": em5}
        for i in range(NCORES)
    ]
    res = run_bass_kernel_spmd(
        nc, in_maps, core_ids=list(range(NCORES)), trace=_trace
    )
    out = np.concatenate([res.results[i]["y"] for i in range(NCORES)], axis=0)
    if _trace:
        _CACHE["last_exec_time_ns"] = res.exec_time_ns
        _CACHE["last_results"] = res
    return out

